# revision 28
# baseline (speedup 1.0000x reference)
"""Trainium2 Bass kernel for grouped vector attention (sparse_attention).

Reference computation (B=2, L1=L2=512, D=256, g=16, n=16):
    Q = x_target @ Wq.T ; K = x_source @ Wk.T ; V = x_source @ Wv.T
    diff = Q.reshape(B,L1,1,n,g) - K.reshape(B,1,L2,n,g)
    scores = relu(einsum('bijng,g->bijn', relu(diff), w_mlp) + b_mlp)
    att = softmax(scores, axis=2)                      # over L2
    out = einsum('bijn,bjgn->bign', att, V.reshape(B,L2,g,n)).reshape(B,L1,D)

Sharding: 8 cores = 2 batches x 4 L2(j)-quarters. Each core handles all 512
queries against its 128 source positions and produces partial (unnormalized)
outputs + partial softmax denominators; the host sums the partials per batch
and divides. Sharding over j (not i) means the exp'd scores come out with j
on partitions - exactly what the att@V contraction needs, so there is no
on-chip transpose anywhere.

Per-core pipeline, "oct" blocks of 8 source positions (16 octs):
  - tmp[d, i] = relu(Q[i,d] - K[j,d]) with d on partitions, i free:
      ScalarE:  activation(Relu, in=QT, bias=-K[:,j], scale=1)
      VectorE:  tensor_scalar(in=QT, s1=-K[:,j], s2=0, op0=add, op1=max)
  - grouped weighted sum over g=16 via TensorE matmul with block-diagonal
    [128 x 32] sel (w_mlp folded).  Four sel variants (2 d-halves x 2 j-
    parities) place the 16 scores of j = 8o+2jj+par at PSUM rows
    32*jj + 16*par + nn, so ALL 128 rows of the score block are live
    (the old quad layout wasted half the rows on zero padding).
  - p = exp(scores + b) off PSUM; pc = max(p, 1)   (= exp(relu(scores+b)))
  - V_sel[o][16*(j-8o) + nn, e] = V[j, e] * (e % 16 == nn)  (built per oct
    by broadcast-DMA from a DRAM copy of V + one masked multiply)
  - out_partial[e, i]  += V_sel[o][:, e-half].T @ pc   (PSUM accumulation
    across all 16 octs);  S_partial[32*(o%4)+nn, i] += ones16.T @ pc
    (col-tiled at tile_position (0, 32*(o%4)); host sums the 4 blocks)
"""

import numpy as np

import concourse.bass as bass
import concourse.bacc as bacc
import concourse.tile as tile
import concourse.mybir as mybir
from concourse.bass_utils import run_bass_kernel_spmd

import ml_dtypes

F32 = mybir.dt.float32
BF16 = mybir.dt.bfloat16
AL = mybir.AluOpType
AF = mybir.ActivationFunctionType

B, L1, L2, D = 2, 512, 512, 256
G = 16           # group size (d_group)
N = 16           # number of groups
NCORES = 8
JSH = 128        # source positions per core (L2 / 4)
NOCT = 16        # 16 octs of 8 source positions
BF = ml_dtypes.bfloat16

# elementwise engine rotation per unit: 0=VectorE, 1=ScalarE
# 71 of 256 units on ScalarE (measured: Scalar unit ~640ns @1x incl 16 exp
# ops it also carries, Vector ~265ns @4x -> balance at x_s ~= 71), spread
# evenly Bresenham-style
N_SCALAR_UNITS = 73
ENGINE_PATTERN = tuple(
    1 if (u * N_SCALAR_UNITS) % 256 + N_SCALAR_UNITS >= 256 else 0
    for u in range(256)
)


def _build(b_val: float):
    """Build + compile the per-core Bass graph. Same graph for all 8 cores."""
    nc = bacc.Bacc(
        "TRN2", target_bir_lowering=False, debug=False, enable_asserts=False
    )

    # ---- DRAM parameters (per-core shards, host-prepped) ----
    xtT_d = nc.dram_tensor("xtT", [2, 128, L1], BF16, kind="ExternalInput")
    xssT_d = nc.dram_tensor("xssT", [2, 128, JSH], BF16, kind="ExternalInput")
    wqT_d = nc.dram_tensor("wqT", [2, 128, D], BF16, kind="ExternalInput")
    wkT_d = nc.dram_tensor("wkT", [2, 128, D], BF16, kind="ExternalInput")
    wvT_d = nc.dram_tensor("wvT", [2, 128, D], BF16, kind="ExternalInput")
    # sel[par][h]: [128, 32], nonzero at col 16*par + d//16
    sel_d = nc.dram_tensor("sel", [2, 2, 128, 32], BF16, kind="ExternalInput")
    vmask_d = nc.dram_tensor("vmask", [128, D], BF16, kind="ExternalInput")
    ones_d = nc.dram_tensor("ones16", [128, N], BF16, kind="ExternalInput")
    outp_d = nc.dram_tensor("outp", [2, 128, L1], BF16, kind="ExternalOutput")
    souts_d = nc.dram_tensor("souts", [128, L1], BF16, kind="ExternalOutput")
    vdram = nc.dram_tensor("vdram", [JSH, D], BF16)

    with tile.TileContext(nc) as tc:
        with (
            tc.tile_pool(name="const", bufs=1) as cpool,
            tc.tile_pool(name="vselp", bufs=1) as vpool,
            tc.tile_pool(name="work", bufs=4) as wpool,
            tc.tile_pool(name="tmps", bufs=12) as tpool,
            tc.tile_pool(name="ps_s", bufs=3, space="PSUM") as ps_pool,
            tc.tile_pool(name="ps_acc", bufs=1, space="PSUM") as pa_pool,
        ):
            # ---- load constants / inputs ----
            xtT = [cpool.tile([128, L1], BF16, name=f"xtT{h}") for h in range(2)]
            xssT = [cpool.tile([128, JSH], BF16, name=f"xssT{h}") for h in range(2)]
            wqT = [cpool.tile([128, D], BF16, name=f"wqT{h}") for h in range(2)]
            wkT = [cpool.tile([128, D], BF16, name=f"wkT{h}") for h in range(2)]
            wvT = [cpool.tile([128, D], BF16, name=f"wvT{h}") for h in range(2)]
            sel = [
                [cpool.tile([128, 32], BF16, name=f"sel{par}{h}") for h in range(2)]
                for par in range(2)
            ]
            vmask = cpool.tile([128, D], BF16, name="vmask")
            ones16 = cpool.tile([128, N], BF16, name="ones16")
            bml = cpool.tile([128, 1], F32, name="bml")

            # ---- accumulators (also the warm-up target: oct 0's V-matmul
            # uses start=True, which clears whatever the warm-up wrote) ----
            ops = [
                pa_pool.tile([128, L1], F32, name=f"ops{eh}") for eh in range(2)
            ]
            sps = pa_pool.tile([128, L1], F32, name="sps")

            # ---- PE warm-up burst: self-contained (memset input), runs
            # during the input-DMA wait so HAM flips toward 8/8; kept short
            # so it does not block the projections behind it in the PE FIFO
            wz = cpool.tile([128, L1], BF16, name="wz")
            nc.vector.memset(wz[:], 0.25)
            for k in range(3):
                nc.tensor.matmul(
                    ops[0][0:32, 0:256],
                    wz[:, 0:32],
                    wz[:, 0:256],
                    start=(k == 0),
                    stop=(k == 2),
                    skip_group_check=True,
                )
            # rows 32k+16..32k+32 of sps are never matmul-written; zero once
            # so the final full-tile evacuation reads defined data (emitted
            # after wz so the warm-up matmuls start as early as possible)
            nc.vector.memset(sps[:], 0.0)
            nc.vector.memset(bml[:], float(b_val))
            # inputs needed by the K/Q projections first; all dma_starts are
            # issued before any ACT-engine op so the strict-FIFO scalar
            # sequencer cannot delay its queue's descriptor writes
            for h in range(2):
                nc.sync.dma_start(xtT[h][:], xtT_d[h])
                nc.scalar.dma_start(wqT[h][:], wqT_d[h])
            for h in range(2):
                nc.sync.dma_start(xssT[h][:], xssT_d[h])
                nc.scalar.dma_start(wkT[h][:], wkT_d[h])
            for h in range(2):
                nc.sync.dma_start(wvT[h][:], wvT_d[h])
                for par in range(2):
                    nc.sync.dma_start(sel[par][h][:], sel_d[par, h])
            nc.scalar.dma_start(vmask[:], vmask_d[:])
            nc.scalar.dma_start(ones16[:], ones_d[:])
            # pre-warm the ACT tables (Relu+Exp, ~1.3us) during the DMA wait
            # instead of paying it at the first real relu unit
            actw = cpool.tile([128, 1], BF16, name="actw")
            nc.scalar.activation(actw[:], wz[:, 0:1], AF.Relu, bias=0.0, scale=1.0)
            nc.scalar.activation(actw[:], wz[:, 0:1], AF.Exp, bias=0.0, scale=1.0)

            # ---- projections on device ----
            # QT[h] (128 e, 512 i) bf16; KTn[h] (128 e, 128 j) f32 negated.
            # eh=0 pair first so h=0 elementwise units can start early.
            QT = [cpool.tile([128, L1], BF16, name=f"QT{h}") for h in range(2)]
            KTn = [cpool.tile([128, JSH], F32, name=f"KTn{h}") for h in range(2)]
            Vt = cpool.tile([128, D], BF16, name="Vt")
            V_sel = [
                vpool.tile([128, D], BF16, name=f"vs{o}") for o in range(NOCT)
            ]

            def build_vsel(o):
                vs = V_sel[o]
                bsrc = (
                    vdram.ap()[8 * o : 8 * o + 8, :]
                    .unsqueeze(1)
                    .broadcast_to((8, 16, D))
                )
                nc.sync.dma_start(vs[:], bsrc)
                nc.vector.tensor_tensor(vs[:], vs[:], vmask[:], op=AL.mult)

            for eh in range(2):
                # Q first: it is the long pole to the first relu units
                # (FD=512 matmuls + a 687ns evacuation vs K's FD=128 + mul)
                psq = ps_pool.tile([128, L1], F32, name="psq", tag="ps_s")
                for dh in range(2):
                    nc.tensor.matmul(
                        psq[:],
                        wqT[dh][:, eh * 128 : (eh + 1) * 128],
                        xtT[dh][:],
                        start=(dh == 0),
                        stop=(dh == 1),
                    )
                nc.scalar.copy(QT[eh][:], psq[:])
                psk = ps_pool.tile([128, JSH], F32, name="psk", tag="psk", bufs=1)
                for dh in range(2):
                    nc.tensor.matmul(
                        psk[:],
                        wkT[dh][:, eh * 128 : (eh + 1) * 128],
                        xssT[dh][:],
                        start=(dh == 0),
                        stop=(dh == 1),
                    )
                nc.scalar.mul(KTn[eh][:], psk[:], -1.0)

            # V projection -> DRAM -> per-oct broadcast+mask; placed after
            # the Q/K projections so the first relu units start as early as
            # possible (V_sel[0] is not consumed until oct 0 finishes)
            psv = ps_pool.tile([128, D], F32, name="psv", tag="psk", bufs=1)
            for dh in range(2):
                nc.tensor.matmul(
                    psv[:],
                    xssT[dh][:],
                    wvT[dh][:],
                    start=(dh == 0),
                    stop=(dh == 1),
                )
            nc.scalar.copy(Vt[:], psv[:])
            nc.sync.dma_start(vdram[:], Vt[:])
            for o in range(6):
                build_vsel(o)

            # ---- main loop: 16 octs of 8 source positions ----
            # Software-pipelined one oct deep: the exp/pc/Vacc consumption of
            # oct o-1 is emitted in the middle of oct o's production, so the
            # strict-FIFO ACT/DVE queues never stall on a just-written PSUM.
            ps_tiles = [None] * NOCT

            def produce(o, part):
                """Emit score units for oct o; part 0 = first half, 1 = rest."""
                if part == 0:
                    if o + 6 < NOCT:
                        build_vsel(o + 6)
                    ps_tiles[o] = ps_pool.tile(
                        [128, L1], F32, name="ps", tag="ps_s"
                    )
                ps = ps_tiles[o]
                for jj in (0, 1) if part == 0 else (2, 3):
                    for par in range(2):
                        j = 8 * o + 2 * jj + par
                        for h in range(2):
                            eng = ENGINE_PATTERN[j * 2 + h]
                            t = tpool.tile([128, L1], BF16, name="t", tag="t")
                            if eng == 1:
                                # t = relu(Q + (-K))
                                nc.scalar.activation(
                                    t[:],
                                    QT[h][:],
                                    AF.Relu,
                                    bias=KTn[h][:, j : j + 1],
                                    scale=1.0,
                                )
                            else:
                                # t = max(Q + (-K), 0)
                                nc.vector.tensor_scalar(
                                    t[:],
                                    QT[h][:],
                                    KTn[h][:, j : j + 1],
                                    0.0,
                                    AL.add,
                                    AL.max,
                                )
                            nc.tensor.matmul(
                                ps[32 * jj : 32 * jj + 32, :],
                                sel[par][h][:],
                                t[:],
                                start=(par == 0 and h == 0),
                                stop=(par == 1 and h == 1),
                                tile_position=(0, 32 * jj),
                            )

            p_tiles = [None] * NOCT

            def consume_exp(o):
                """p = exp(scores + b) off oct o's finished PSUM block."""
                p_tiles[o] = wpool.tile([128, L1], BF16, name="p", tag="p", bufs=6)
                nc.scalar.activation(
                    p_tiles[o][:], ps_tiles[o][:], AF.Exp, bias=bml[:], scale=1.0
                )

            def consume_rest(o):
                """pc = max(p, 1) = exp(relu(scores+b)); accumulation matmuls."""
                pc = wpool.tile([128, L1], BF16, name="pc", tag="pc", bufs=12)
                nc.vector.tensor_scalar(pc[:], p_tiles[o][:], 1.0, None, AL.max)
                for eh in range(2):
                    nc.tensor.matmul(
                        ops[eh][:],
                        V_sel[o][:, eh * 128 : (eh + 1) * 128],
                        pc[:],
                        start=(o == 0),
                        stop=(o == NOCT - 1),
                        skip_group_check=True,
                    )
                k4 = o % 4
                nc.tensor.matmul(
                    sps[32 * k4 : 32 * k4 + N, :],
                    ones16[:, 0:N],
                    pc[:],
                    start=(o < 4),
                    stop=(o >= NOCT - 4),
                    skip_group_check=True,
                    tile_position=(0, 32 * k4),
                )

            produce(0, 0)
            produce(0, 1)
            for o in range(1, NOCT):
                produce(o, 0)
                consume_exp(o - 1)
                produce(o, 1)
                consume_rest(o - 1)
            consume_exp(NOCT - 1)
            consume_rest(NOCT - 1)

            # ---- evacuate + store (outputs split across both HWDGE queues;
            # souts last since the sps accumulator stops latest) ----
            for eh in range(2):
                ou = wpool.tile([128, L1], BF16, name="ou", tag="ou", bufs=2)
                if eh == 0:
                    nc.vector.tensor_copy(ou[:], ops[eh][:])
                    nc.scalar.dma_start(outp_d[eh], ou[:])
                else:
                    nc.scalar.copy(ou[:], ops[eh][:])
                    nc.sync.dma_start(outp_d[eh], ou[:])
            so = wpool.tile([128, L1], BF16, name="so")
            nc.vector.tensor_copy(so[:], sps[:])
            nc.sync.dma_start(souts_d[:], so[:])

    nc.compile()
    return nc


_CACHE: dict = {}


def _get_graph(b_val: float):
    key = round(float(b_val), 10)
    if key not in _CACHE:
        _CACHE[key] = _build(float(b_val))
    return _CACHE[key]


def _host_prep(x_source, x_target, Wq, Wk, Wv, w_mlp):
    """Build per-core input maps (numpy, bf16)."""
    w_full = np.tile(np.asarray(w_mlp, np.float32), D // G)  # w_full[d] = w[d%16]
    # sel[par][h]: nonzero at col 16*par + d//16 (d = 128h + dl)
    sel = np.zeros((2, 2, 128, 32), np.float32)
    for par in range(2):
        for h in range(2):
            for dl in range(128):
                d = 128 * h + dl
                sel[par, h, dl, 16 * par + d // G] = w_full[d]
    # V_sel mask: row p = 16*(j-8o) + nn, col e: keep if e%16 == nn = p%16
    vmask = np.zeros((128, D), np.float32)
    for p in range(128):
        vmask[p, (p % 16) :: G] = 1.0
    # S selector: row p -> column p%16
    ones16 = np.zeros((128, N), np.float32)
    for p in range(128):
        ones16[p, p % 16] = 1.0

    def split_h(a):  # (256, X) -> (2, 128, X)
        return np.ascontiguousarray(a.reshape(2, 128, a.shape[1]))

    wq_b = split_h(np.asarray(Wq, np.float32).T).astype(BF)
    wk_b = split_h(np.asarray(Wk, np.float32).T).astype(BF)
    wv_b = split_h(np.asarray(Wv, np.float32).T).astype(BF)
    sel_b = sel.astype(BF)
    vmask_b = vmask.astype(BF)
    ones_b = ones16.astype(BF)

    xtT = [
        split_h(np.asarray(x_target[b], np.float32).T).astype(BF) for b in range(B)
    ]
    xsT = [np.asarray(x_source[b], np.float32).T for b in range(B)]
    in_maps = []
    for core in range(NCORES):
        b, jq = divmod(core, 4)
        j0 = jq * JSH
        xssT = split_h(xsT[b][:, j0 : j0 + JSH]).astype(BF)
        in_maps.append(
            {
                "xtT": xtT[b],
                "xssT": xssT,
                "wqT": wq_b,
                "wkT": wk_b,
                "wvT": wv_b,
                "sel": sel_b,
                "vmask": vmask_b,
                "ones16": ones_b,
            }
        )
    return in_maps


def _host_gather(results):
    """Sum partials over j-shards, normalize, reshape to (B, L1, D)."""
    out = np.empty((B, L1, D), np.float32)
    for b in range(B):
        cores = [b * 4 + jq for jq in range(4)]
        U = sum(
            results[c]["outp"].reshape(D, L1).astype(np.float64) for c in cores
        )  # (e, i)
        # souts rows 32k..32k+16 hold partial denominators; sum the 4 blocks
        S = sum(
            results[c]["souts"]
            .astype(np.float64)
            .reshape(4, 32, L1)[:, :N, :]
            .sum(axis=0)
            for c in cores
        )  # (nn, i)
        att = U / S[np.arange(D) % N, :]  # (e, i)
        out[b] = att.T.astype(np.float32)
    return out


def run(inputs, trace=False, **kwargs):
    nc = _get_graph(float(np.asarray(inputs["b_mlp"]).reshape(-1)[0]))
    in_maps = _host_prep(
        inputs["x_source"],
        inputs["x_target"],
        inputs["Wq"],
        inputs["Wk"],
        inputs["Wv"],
        inputs["w_mlp"],
    )
    res = run_bass_kernel_spmd(
        nc, in_maps, core_ids=list(range(NCORES)), trace=trace, **kwargs
    )
    return _host_gather(res.results), res


def kernel(**inputs) -> np.ndarray:
    out, _ = run(inputs, trace=False)
    return out


# revision 29
# speedup vs baseline: 1.0134x; 1.0134x over previous
"""Trainium2 Bass kernel for grouped vector attention (sparse_attention).

Reference computation (B=2, L1=L2=512, D=256, g=16, n=16):
    Q = x_target @ Wq.T ; K = x_source @ Wk.T ; V = x_source @ Wv.T
    diff = Q.reshape(B,L1,1,n,g) - K.reshape(B,1,L2,n,g)
    scores = relu(einsum('bijng,g->bijn', relu(diff), w_mlp) + b_mlp)
    att = softmax(scores, axis=2)                      # over L2
    out = einsum('bijn,bjgn->bign', att, V.reshape(B,L2,g,n)).reshape(B,L1,D)

Sharding: 8 cores = 2 batches x 4 L2(j)-quarters. Each core handles all 512
queries against its 128 source positions and produces partial (unnormalized)
outputs + partial softmax denominators; the host sums the partials per batch
and divides. Sharding over j (not i) means the exp'd scores come out with j
on partitions - exactly what the att@V contraction needs, so there is no
on-chip transpose anywhere.

Per-core pipeline, "oct" blocks of 8 source positions (16 octs):
  - tmp[d, i] = relu(Q[i,d] - K[j,d]) with d on partitions, i free:
      ScalarE:  activation(Relu, in=QT, bias=-K[:,j], scale=1)
      VectorE:  tensor_scalar(in=QT, s1=-K[:,j], s2=0, op0=add, op1=max)
  - grouped weighted sum over g=16 via TensorE matmul with block-diagonal
    [128 x 32] sel (w_mlp folded).  Four sel variants (2 d-halves x 2 j-
    parities) place the 16 scores of j = 8o+2jj+par at PSUM rows
    32*jj + 16*par + nn, so ALL 128 rows of the score block are live
    (the old quad layout wasted half the rows on zero padding).
  - p = exp(scores + b) off PSUM; pc = max(p, 1)   (= exp(relu(scores+b)))
  - V_sel[o][16*(j-8o) + nn, e] = V[j, e] * (e % 16 == nn)  (built per oct
    by broadcast-DMA from a DRAM copy of V + one masked multiply)
  - out_partial[e, i]  += V_sel[o][:, e-half].T @ pc   (PSUM accumulation
    across all 16 octs);  S_partial[32*(o%4)+nn, i] += ones16.T @ pc
    (col-tiled at tile_position (0, 32*(o%4)); host sums the 4 blocks)
"""

import numpy as np

import concourse.bass as bass
import concourse.bacc as bacc
import concourse.tile as tile
import concourse.mybir as mybir
from concourse.bass_utils import run_bass_kernel_spmd

import ml_dtypes

F32 = mybir.dt.float32
BF16 = mybir.dt.bfloat16
AL = mybir.AluOpType
AF = mybir.ActivationFunctionType

B, L1, L2, D = 2, 512, 512, 256
G = 16           # group size (d_group)
N = 16           # number of groups
NCORES = 8
JSH = 128        # source positions per core (L2 / 4)
NOCT = 16        # 16 octs of 8 source positions
BF = ml_dtypes.bfloat16

# elementwise engine rotation per unit: 0=VectorE, 1=ScalarE
# 71 of 256 units on ScalarE (measured: Scalar unit ~640ns @1x incl 16 exp
# ops it also carries, Vector ~265ns @4x -> balance at x_s ~= 71), spread
# evenly Bresenham-style
N_SCALAR_UNITS = 71
ENGINE_PATTERN = tuple(
    1 if (u * N_SCALAR_UNITS) % 256 + N_SCALAR_UNITS >= 256 else 0
    for u in range(256)
)


def _build(b_val: float):
    """Build + compile the per-core Bass graph. Same graph for all 8 cores."""
    nc = bacc.Bacc(
        "TRN2", target_bir_lowering=False, debug=False, enable_asserts=False
    )

    # ---- DRAM parameters (per-core shards, host-prepped) ----
    xtT_d = nc.dram_tensor("xtT", [2, 128, L1], BF16, kind="ExternalInput")
    xssT_d = nc.dram_tensor("xssT", [2, 128, JSH], BF16, kind="ExternalInput")
    wqT_d = nc.dram_tensor("wqT", [2, 128, D], BF16, kind="ExternalInput")
    wkT_d = nc.dram_tensor("wkT", [2, 128, D], BF16, kind="ExternalInput")
    wvT_d = nc.dram_tensor("wvT", [2, 128, D], BF16, kind="ExternalInput")
    # sel[par][h]: [128, 32], nonzero at col 16*par + d//16
    sel_d = nc.dram_tensor("sel", [2, 2, 128, 32], BF16, kind="ExternalInput")
    vmask_d = nc.dram_tensor("vmask", [128, D], BF16, kind="ExternalInput")
    ones_d = nc.dram_tensor("ones16", [128, N], BF16, kind="ExternalInput")
    outp_d = nc.dram_tensor("outp", [2, 128, L1], BF16, kind="ExternalOutput")
    souts_d = nc.dram_tensor("souts", [128, L1], BF16, kind="ExternalOutput")
    vdram = nc.dram_tensor("vdram", [JSH, D], BF16)

    with tile.TileContext(nc) as tc:
        with (
            tc.tile_pool(name="const", bufs=1) as cpool,
            tc.tile_pool(name="vselp", bufs=1) as vpool,
            tc.tile_pool(name="work", bufs=4) as wpool,
            tc.tile_pool(name="tmps", bufs=12) as tpool,
            tc.tile_pool(name="ps_s", bufs=3, space="PSUM") as ps_pool,
            tc.tile_pool(name="ps_acc", bufs=1, space="PSUM") as pa_pool,
        ):
            # ---- load constants / inputs ----
            xtT = [cpool.tile([128, L1], BF16, name=f"xtT{h}") for h in range(2)]
            xssT = [cpool.tile([128, JSH], BF16, name=f"xssT{h}") for h in range(2)]
            wqT = [cpool.tile([128, D], BF16, name=f"wqT{h}") for h in range(2)]
            wkT = [cpool.tile([128, D], BF16, name=f"wkT{h}") for h in range(2)]
            wvT = [cpool.tile([128, D], BF16, name=f"wvT{h}") for h in range(2)]
            sel = [
                [cpool.tile([128, 32], BF16, name=f"sel{par}{h}") for h in range(2)]
                for par in range(2)
            ]
            vmask = cpool.tile([128, D], BF16, name="vmask")
            ones16 = cpool.tile([128, N], BF16, name="ones16")
            bml = cpool.tile([128, 1], F32, name="bml")

            # ---- accumulators (also the warm-up target: oct 0's V-matmul
            # uses start=True, which clears whatever the warm-up wrote) ----
            ops = [
                pa_pool.tile([128, L1], F32, name=f"ops{eh}") for eh in range(2)
            ]
            sps = pa_pool.tile([128, L1], F32, name="sps")

            # ---- PE warm-up burst: self-contained (memset input), runs
            # during the input-DMA wait so HAM flips toward 8/8; kept short
            # so it does not block the projections behind it in the PE FIFO
            wz = cpool.tile([128, L1], BF16, name="wz")
            nc.vector.memset(wz[:], 0.25)
            for k in range(3):
                nc.tensor.matmul(
                    ops[0][0:32, 0:256],
                    wz[:, 0:32],
                    wz[:, 0:256],
                    start=(k == 0),
                    stop=(k == 2),
                    skip_group_check=True,
                )
            # rows 32k+16..32k+32 of sps are never matmul-written; zero once
            # so the final full-tile evacuation reads defined data (emitted
            # after wz so the warm-up matmuls start as early as possible)
            nc.vector.memset(sps[:], 0.0)
            nc.vector.memset(bml[:], float(b_val))
            # inputs needed by the K/Q projections first; all dma_starts are
            # issued before any ACT-engine op so the strict-FIFO scalar
            # sequencer cannot delay its queue's descriptor writes
            for h in range(2):
                nc.sync.dma_start(xtT[h][:], xtT_d[h])
                nc.scalar.dma_start(wqT[h][:], wqT_d[h])
            for h in range(2):
                nc.sync.dma_start(xssT[h][:], xssT_d[h])
                nc.scalar.dma_start(wkT[h][:], wkT_d[h])
            for h in range(2):
                nc.sync.dma_start(wvT[h][:], wvT_d[h])
                for par in range(2):
                    nc.sync.dma_start(sel[par][h][:], sel_d[par, h])
            nc.scalar.dma_start(vmask[:], vmask_d[:])
            nc.scalar.dma_start(ones16[:], ones_d[:])
            # pre-warm the ACT tables (Relu+Exp, ~1.3us) during the DMA wait
            # instead of paying it at the first real relu unit
            actw = cpool.tile([128, 1], BF16, name="actw")
            nc.scalar.activation(actw[:], wz[:, 0:1], AF.Relu, bias=0.0, scale=1.0)
            nc.scalar.activation(actw[:], wz[:, 0:1], AF.Exp, bias=0.0, scale=1.0)

            # ---- projections on device ----
            # QT[h] (128 e, 512 i) bf16; KTn[h] (128 e, 128 j) f32 negated.
            # eh=0 pair first so h=0 elementwise units can start early.
            QT = [cpool.tile([128, L1], BF16, name=f"QT{h}") for h in range(2)]
            KTn = [cpool.tile([128, JSH], F32, name=f"KTn{h}") for h in range(2)]
            Vt = cpool.tile([128, D], BF16, name="Vt")
            V_sel = [
                vpool.tile([128, D], BF16, name=f"vs{o}") for o in range(NOCT)
            ]

            def build_vsel(o):
                vs = V_sel[o]
                bsrc = (
                    vdram.ap()[8 * o : 8 * o + 8, :]
                    .unsqueeze(1)
                    .broadcast_to((8, 16, D))
                )
                nc.sync.dma_start(vs[:], bsrc)
                nc.vector.tensor_tensor(vs[:], vs[:], vmask[:], op=AL.mult)

            for eh in range(2):
                # Q first: it is the long pole to the first relu units
                # (FD=512 matmuls + a 687ns evacuation vs K's FD=128 + mul)
                psq = ps_pool.tile([128, L1], F32, name="psq", tag="ps_s")
                for dh in range(2):
                    nc.tensor.matmul(
                        psq[:],
                        wqT[dh][:, eh * 128 : (eh + 1) * 128],
                        xtT[dh][:],
                        start=(dh == 0),
                        stop=(dh == 1),
                    )
                nc.scalar.copy(QT[eh][:], psq[:])
                psk = ps_pool.tile([128, JSH], F32, name="psk", tag="psk", bufs=1)
                for dh in range(2):
                    nc.tensor.matmul(
                        psk[:],
                        wkT[dh][:, eh * 128 : (eh + 1) * 128],
                        xssT[dh][:],
                        start=(dh == 0),
                        stop=(dh == 1),
                    )
                nc.scalar.mul(KTn[eh][:], psk[:], -1.0)

            # V projection -> DRAM -> per-oct broadcast+mask; placed after
            # the Q/K projections so the first relu units start as early as
            # possible (V_sel[0] is not consumed until oct 0 finishes)
            psv = ps_pool.tile([128, D], F32, name="psv", tag="psk", bufs=1)
            for dh in range(2):
                nc.tensor.matmul(
                    psv[:],
                    xssT[dh][:],
                    wvT[dh][:],
                    start=(dh == 0),
                    stop=(dh == 1),
                )
            nc.scalar.copy(Vt[:], psv[:])
            nc.sync.dma_start(vdram[:], Vt[:])
            for o in range(6):
                build_vsel(o)

            # ---- main loop: 16 octs of 8 source positions ----
            # Software-pipelined one oct deep: the exp/pc/Vacc consumption of
            # oct o-1 is emitted in the middle of oct o's production, so the
            # strict-FIFO ACT/DVE queues never stall on a just-written PSUM.
            ps_tiles = [None] * NOCT

            def produce(o, part):
                """Emit score units for oct o; part 0 = first half, 1 = rest."""
                if part == 0:
                    if o + 6 < NOCT:
                        build_vsel(o + 6)
                    ps_tiles[o] = ps_pool.tile(
                        [128, L1], F32, name="ps", tag="ps_s"
                    )
                ps = ps_tiles[o]
                for jj in (0, 1) if part == 0 else (2, 3):
                    for par in range(2):
                        j = 8 * o + 2 * jj + par
                        for h in range(2):
                            eng = ENGINE_PATTERN[j * 2 + h]
                            t = tpool.tile([128, L1], BF16, name="t", tag="t")
                            if eng == 1:
                                # t = relu(Q + (-K))
                                nc.scalar.activation(
                                    t[:],
                                    QT[h][:],
                                    AF.Relu,
                                    bias=KTn[h][:, j : j + 1],
                                    scale=1.0,
                                )
                            else:
                                # t = max(Q + (-K), 0)
                                nc.vector.tensor_scalar(
                                    t[:],
                                    QT[h][:],
                                    KTn[h][:, j : j + 1],
                                    0.0,
                                    AL.add,
                                    AL.max,
                                )
                            nc.tensor.matmul(
                                ps[32 * jj : 32 * jj + 32, :],
                                sel[par][h][:],
                                t[:],
                                start=(par == 0 and h == 0),
                                stop=(par == 1 and h == 1),
                                tile_position=(0, 32 * jj),
                            )

            p_tiles = [None] * NOCT

            def consume_exp(o):
                """p = exp(scores + b) off oct o's finished PSUM block."""
                p_tiles[o] = wpool.tile([128, L1], BF16, name="p", tag="p", bufs=6)
                nc.scalar.activation(
                    p_tiles[o][:], ps_tiles[o][:], AF.Exp, bias=bml[:], scale=1.0
                )

            def consume_rest(o):
                """pc = max(p, 1) = exp(relu(scores+b)); accumulation matmuls."""
                pc = wpool.tile([128, L1], BF16, name="pc", tag="pc", bufs=12)
                nc.vector.tensor_scalar(pc[:], p_tiles[o][:], 1.0, None, AL.max)
                for eh in range(2):
                    nc.tensor.matmul(
                        ops[eh][:],
                        V_sel[o][:, eh * 128 : (eh + 1) * 128],
                        pc[:],
                        start=(o == 0),
                        stop=(o == NOCT - 1),
                        skip_group_check=True,
                    )
                k4 = o % 4
                nc.tensor.matmul(
                    sps[32 * k4 : 32 * k4 + N, :],
                    ones16[:, 0:N],
                    pc[:],
                    start=(o < 4),
                    stop=(o >= NOCT - 4),
                    skip_group_check=True,
                    tile_position=(0, 32 * k4),
                )

            produce(0, 0)
            produce(0, 1)
            for o in range(1, NOCT):
                produce(o, 0)
                consume_exp(o - 1)
                produce(o, 1)
                consume_rest(o - 1)
            consume_exp(NOCT - 1)
            consume_rest(NOCT - 1)

            # ---- evacuate + store (outputs split across both HWDGE queues;
            # souts last since the sps accumulator stops latest) ----
            for eh in range(2):
                ou = wpool.tile([128, L1], BF16, name="ou", tag="ou", bufs=2)
                if eh == 0:
                    nc.vector.tensor_copy(ou[:], ops[eh][:])
                    nc.scalar.dma_start(outp_d[eh], ou[:])
                else:
                    nc.scalar.copy(ou[:], ops[eh][:])
                    nc.sync.dma_start(outp_d[eh], ou[:])
            so = wpool.tile([128, L1], BF16, name="so")
            nc.vector.tensor_copy(so[:], sps[:])
            nc.sync.dma_start(souts_d[:], so[:])

    nc.compile()
    return nc


_CACHE: dict = {}


def _get_graph(b_val: float):
    key = round(float(b_val), 10)
    if key not in _CACHE:
        _CACHE[key] = _build(float(b_val))
    return _CACHE[key]


def _host_prep(x_source, x_target, Wq, Wk, Wv, w_mlp):
    """Build per-core input maps (numpy, bf16)."""
    w_full = np.tile(np.asarray(w_mlp, np.float32), D // G)  # w_full[d] = w[d%16]
    # sel[par][h]: nonzero at col 16*par + d//16 (d = 128h + dl)
    sel = np.zeros((2, 2, 128, 32), np.float32)
    for par in range(2):
        for h in range(2):
            for dl in range(128):
                d = 128 * h + dl
                sel[par, h, dl, 16 * par + d // G] = w_full[d]
    # V_sel mask: row p = 16*(j-8o) + nn, col e: keep if e%16 == nn = p%16
    vmask = np.zeros((128, D), np.float32)
    for p in range(128):
        vmask[p, (p % 16) :: G] = 1.0
    # S selector: row p -> column p%16
    ones16 = np.zeros((128, N), np.float32)
    for p in range(128):
        ones16[p, p % 16] = 1.0

    def split_h(a):  # (256, X) -> (2, 128, X)
        return np.ascontiguousarray(a.reshape(2, 128, a.shape[1]))

    wq_b = split_h(np.asarray(Wq, np.float32).T).astype(BF)
    wk_b = split_h(np.asarray(Wk, np.float32).T).astype(BF)
    wv_b = split_h(np.asarray(Wv, np.float32).T).astype(BF)
    sel_b = sel.astype(BF)
    vmask_b = vmask.astype(BF)
    ones_b = ones16.astype(BF)

    xtT = [
        split_h(np.asarray(x_target[b], np.float32).T).astype(BF) for b in range(B)
    ]
    xsT = [np.asarray(x_source[b], np.float32).T for b in range(B)]
    in_maps = []
    for core in range(NCORES):
        b, jq = divmod(core, 4)
        j0 = jq * JSH
        xssT = split_h(xsT[b][:, j0 : j0 + JSH]).astype(BF)
        in_maps.append(
            {
                "xtT": xtT[b],
                "xssT": xssT,
                "wqT": wq_b,
                "wkT": wk_b,
                "wvT": wv_b,
                "sel": sel_b,
                "vmask": vmask_b,
                "ones16": ones_b,
            }
        )
    return in_maps


def _host_gather(results):
    """Sum partials over j-shards, normalize, reshape to (B, L1, D)."""
    out = np.empty((B, L1, D), np.float32)
    for b in range(B):
        cores = [b * 4 + jq for jq in range(4)]
        U = sum(
            results[c]["outp"].reshape(D, L1).astype(np.float64) for c in cores
        )  # (e, i)
        # souts rows 32k..32k+16 hold partial denominators; sum the 4 blocks
        S = sum(
            results[c]["souts"]
            .astype(np.float64)
            .reshape(4, 32, L1)[:, :N, :]
            .sum(axis=0)
            for c in cores
        )  # (nn, i)
        att = U / S[np.arange(D) % N, :]  # (e, i)
        out[b] = att.T.astype(np.float32)
    return out


def run(inputs, trace=False, **kwargs):
    nc = _get_graph(float(np.asarray(inputs["b_mlp"]).reshape(-1)[0]))
    in_maps = _host_prep(
        inputs["x_source"],
        inputs["x_target"],
        inputs["Wq"],
        inputs["Wk"],
        inputs["Wv"],
        inputs["w_mlp"],
    )
    res = run_bass_kernel_spmd(
        nc, in_maps, core_ids=list(range(NCORES)), trace=trace, **kwargs
    )
    return _host_gather(res.results), res


def kernel(**inputs) -> np.ndarray:
    out, _ = run(inputs, trace=False)
    return out
